# revision 7
# baseline (speedup 1.0000x reference)
"""Trainium2 Bass kernel for 2-layer GAT — remote_dma AllGather variant.

Differences from the ncfw-collective baseline (kernel.py):
  - The per-layer AllGather of hx rows (h bf16[768] + a_s fp32, 770 bf16 cols
    transmitted) is done with remote_dma_broadcast: 7 per-slot SBUF->SBUF
    sends in XOR pairing (slot s on core c holds rows of core c^s), all 14
    engine-lanes concurrently, instead of a blocking ncfw collective.
  - Received slots (plus own rows) are spilled SBUF->DRAM into hx_full
    [8*1280, 896] (896-col rows keep dma_gather's 256B elem alignment);
    phase D gathers from DRAM exactly as before with slot-remapped indices.
  - Landing-buffer reuse across layers/reps is guarded by a remote-sem ack
    protocol (wait_ge targets are monotonic; NRT zeroes user sems at the
    start of every execution).
"""

import os
import sys
from contextlib import ExitStack

import numpy as np

for _p in ("/opt/trn_rl_repo", "/root/.axon_site/_ro/trn_rl_repo"):
    if os.path.isdir(_p) and _p not in sys.path:
        sys.path.insert(0, _p)

import ml_dtypes  # noqa: E402

import concourse.bass as bass  # noqa: E402
import concourse.tile as tile  # noqa: E402
from concourse import bacc, mybir  # noqa: E402
from concourse.bass_utils import run_bass_kernel_spmd  # noqa: E402
from concourse.masks import make_identity  # noqa: E402

F32 = mybir.dt.float32
BF16 = mybir.dt.bfloat16
I16 = mybir.dt.int16

N_NODES = 10000
DIM = 768
N_CORES = 8
SHARD = N_NODES // N_CORES  # 1250
P = 128
N_RANGES = (SHARD + P - 1) // P  # 10
SLOT_ROWS = N_RANGES * P  # 1280 rows per slot in hx_full
ROW = 896  # bf16 cols per hx_full DRAM row (1792B, mult of 256 for gather)
TXCOL = DIM + 4  # 772 bf16 cols: h[768] | a_s fp32 | const 1.0 | pad
ACOL = DIM  # a_s fp32 lives at bf16 cols [768:770]
NEG_SLOPE = 0.2
PENALTY = -3000.0
CHUNK_T = 6  # edge tiles per dma_gather chunk
# Logical core -> physical TPB id (probed on HW; remote_dma XOR routing acts
# on physical ids). Slot of owner o's rows on core c = PHI[o] ^ PHI[c].
# Default from probing this container; _probe_phi() re-derives it at runtime.
PHI = np.array([0, 1, 2, 3, 6, 7, 4, 5])


def _probe_phi():
    """Discover the XOR-slot pairing by broadcasting each core's id once."""
    nc = bacc.Bacc(
        "TRN2", target_bir_lowering=False, debug=False, num_devices=N_CORES
    )
    myid_d = nc.dram_tensor("myid", [P, 16], F32, kind="ExternalInput")
    out_d = nc.dram_tensor("probe_out", [P, N_CORES * 16], F32, kind="ExternalOutput")
    arr_sem = nc.alloc_semaphore("p_arr")
    loc_sem = nc.alloc_semaphore("p_loc")
    prep_sem = nc.alloc_semaphore("p_prep")
    with tile.TileContext(nc) as tc, ExitStack() as ctx:
        cp = ctx.enter_context(tc.tile_pool(name="c", bufs=1))
        self_t = cp.tile([P, 16], F32, tag="selft")
        land = cp.tile([P, N_CORES - 1, 16], F32, tag="landt")
        outt = cp.tile([P, N_CORES, 16], F32, tag="outt")
        nc.sync.dma_start(out=self_t[:], in_=myid_d[:])
        with tc.tile_critical(name="probe"):
            g = nc.gpsimd
            for s in range(1, N_CORES):
                rdests = [(0, s) if k == s else None for k in range(N_CORES)]
                g.remote_dma_broadcast(
                    out_ap=land[:, s - 1], in_ap=self_t[:],
                    remote_sem=arr_sem, local_sem=loc_sem, rdests=rdests,
                ).then_inc(prep_sem, 1)
            g.wait_ge(prep_sem, 7)
            g.trigger_dma(count=7)
            g.wait_ge(arr_sem, 14)
            g.wait_ge(loc_sem, 112)
        nc.vector.tensor_copy(out=outt[:, 0, :], in_=self_t[:])
        nc.vector.tensor_copy(out=outt[:, 1:, :], in_=land[:])
        nc.sync.dma_start(out=out_d[:], in_=outt[:])
    nc.compile()
    in_maps = [{"myid": np.full((P, 16), float(c), np.float32)} for c in range(N_CORES)]
    res = run_bass_kernel_spmd(nc, in_maps, list(range(N_CORES)))
    M = np.zeros((N_CORES, N_CORES), int)
    for c in range(N_CORES):
        o = res.results[c]["probe_out"].reshape(P, N_CORES, 16)
        for s in range(N_CORES):
            vals = np.unique(o[:, s, :])
            M[c, s] = int(vals[0]) if len(vals) == 1 else -1
    for phi0 in range(N_CORES):
        phi = {0: phi0}
        for s in range(1, N_CORES):
            if M[0, s] < 0:
                break
            phi[M[0, s]] = phi0 ^ s
        if len(phi) == N_CORES and all(
            M[c, s] >= 0 and phi[M[c, s]] == (phi[c] ^ s)
            for c in range(N_CORES)
            for s in range(N_CORES)
        ):
            return np.array([phi[i] for i in range(N_CORES)])
    raise RuntimeError(f"no XOR-consistent mapping: {M}")


def _range_rows(r):
    return min(P, SHARD - r * P)


# ---------------------------------------------------------------------------
# host preprocessing
# ---------------------------------------------------------------------------


def preprocess(x, edge_index, W1, att_src1, att_dst1, W2, att_src2, att_dst2):
    n = x.shape[0]
    src = np.concatenate([np.asarray(edge_index[0]), np.arange(n, dtype=np.int64)])
    dst = np.concatenate([np.asarray(edge_index[1]), np.arange(n, dtype=np.int64)])

    core_of = dst // SHARD
    buckets = [[None] * N_RANGES for _ in range(N_CORES)]
    for c in range(N_CORES):
        sel = core_of == c
        s_c = src[sel]
        d_c = dst[sel] - c * SHARD
        rid = d_c // P
        # per-core gather index: slot = PHI[owner] ^ PHI[c], row = src % SHARD
        gidx_c = (PHI[s_c // SHARD] ^ PHI[c]) * SLOT_ROWS + (s_c % SHARD)
        for r in range(N_RANGES):
            m = rid == r
            g_r = gidx_c[m]
            rel_r = (d_c[m] - r * P).astype(np.int64)
            order = np.argsort(g_r, kind="stable")  # gather locality
            buckets[c][r] = (g_r[order], rel_r[order])

    tiles_per_range = [
        max(1, max((len(buckets[c][r][0]) + P - 1) // P for c in range(N_CORES)))
        for r in range(N_RANGES)
    ]
    total_tiles = sum(tiles_per_range)
    total_slots = total_tiles * P

    Wv1 = np.concatenate(
        [W1, (W1 @ att_src1)[:, None], (W1 @ att_dst1)[:, None]], axis=1
    ).astype(np.float32)
    Wv2 = np.concatenate(
        [W2, (W2 @ att_src2)[:, None], (W2 @ att_dst2)[:, None]], axis=1
    ).astype(np.float32)

    in_maps = []
    for c in range(N_CORES):
        idx_slots = np.zeros(total_slots, dtype=np.int16)
        rel_slots = np.full(total_slots, -1, dtype=np.int32)
        off = 0
        for r in range(N_RANGES):
            g_r, rel_r = buckets[c][r]
            k = len(g_r)
            idx_slots[off : off + k] = g_r.astype(np.int16)
            rel_slots[off : off + k] = rel_r
            off += tiles_per_range[r] * P
        idx16 = idx_slots.reshape(-1, 16).T
        idx16 = np.tile(idx16, (8, 1)).copy()
        rel = rel_slots.reshape(total_tiles, P)
        pen = np.where(
            rel[:, :, None] == np.arange(P)[None, None, :], 0.0, PENALTY
        )
        pen = np.ascontiguousarray(pen.transpose(1, 0, 2).reshape(P, total_tiles * P))
        xT = np.zeros((DIM, N_RANGES * P), dtype=ml_dtypes.bfloat16)
        xT[:, :SHARD] = np.asarray(x[c * SHARD : (c + 1) * SHARD]).T.astype(
            ml_dtypes.bfloat16
        )
        in_maps.append(
            {
                "xT": xT,
                "Wv1": Wv1.astype(ml_dtypes.bfloat16),
                "Wv2": Wv2.astype(ml_dtypes.bfloat16),
                "idx": idx16.astype(np.int16),
                "pen": pen.astype(ml_dtypes.bfloat16),
            }
        )
    return in_maps, tiles_per_range


# ---------------------------------------------------------------------------
# device program
# ---------------------------------------------------------------------------


def build_program(tiles_per_range, repeat=1, variant="full", ng=5):
    total_tiles = sum(tiles_per_range)
    total_slots = total_tiles * P

    nc = bacc.Bacc(
        "TRN2",
        target_bir_lowering=False,
        debug=False,
        num_devices=N_CORES,
    )

    xT_d = nc.dram_tensor("xT", [DIM, N_RANGES * P], BF16, kind="ExternalInput")
    Wv1_d = nc.dram_tensor("Wv1", [DIM, DIM + 2], BF16, kind="ExternalInput")
    Wv2_d = nc.dram_tensor("Wv2", [DIM, DIM + 2], BF16, kind="ExternalInput")
    idx_d = nc.dram_tensor("idx", [P, total_slots // 16], I16, kind="ExternalInput")
    pen_d = nc.dram_tensor("pen", [P, total_tiles * P], BF16, kind="ExternalInput")
    out_d = nc.dram_tensor("out", [SHARD, DIM], F32, kind="ExternalOutput")

    hx_full = [
        nc.dram_tensor(f"hx{L}_full", [N_CORES * SLOT_ROWS, ROW], BF16)
        for L in (1, 2)
    ]
    h1pad = nc.dram_tensor("h1pad", [N_RANGES * P, DIM], BF16)

    KT = DIM // P  # 6 k-tiles

    arr_sems = [nc.alloc_semaphore(f"rd_arr{s}") for s in range(1, N_CORES)]
    loc_sem = nc.alloc_semaphore("rd_loc")
    prep_sem = nc.alloc_semaphore("rd_prep")
    spill_sem = nc.alloc_semaphore("rd_spill")
    ack_sem = nc.alloc_semaphore("rd_ack")

    with tile.TileContext(nc) as tc, ExitStack() as ctx:
        const_p = ctx.enter_context(tc.tile_pool(name="const", bufs=1))
        sb = ctx.enter_context(tc.tile_pool(name="sb", bufs=3))
        sb2 = ctx.enter_context(tc.tile_pool(name="sb2", bufs=2))
        gp = ctx.enter_context(tc.tile_pool(name="gath", bufs=2))
        pp = ctx.enter_context(tc.tile_pool(name="psum", bufs=2, space="PSUM"))
        ppA = ctx.enter_context(tc.tile_pool(name="psumA", bufs=1, space="PSUM"))
        ppT = ctx.enter_context(tc.tile_pool(name="psumT", bufs=2, space="PSUM"))

        # resident constants
        identity = const_p.tile([P, P], F32)
        make_identity(nc, identity[:])
        ones_col = const_p.tile([P, 1], BF16)
        nc.vector.memset(ones_col[:], 1.0)
        idx_sb = const_p.tile([P, total_slots // 16], I16)
        nc.sync.dma_start(out=idx_sb[:], in_=idx_d[:])
        Wv_sb = [
            const_p.tile([P, KT, DIM + 2], BF16, tag="wv0", name="wv0"),
            const_p.tile([P, KT, DIM + 2], BF16, tag="wv1", name="wv1"),
        ]
        for L, wd in enumerate((Wv1_d, Wv2_d)):
            for k in range(KT):
                nc.sync.dma_start(out=Wv_sb[L][:, k, :], in_=wd[k * P : (k + 1) * P, :])
        adcol = [
            const_p.tile([P, N_RANGES], F32, tag=f"ad{L}", name=f"adcol{L}")
            for L in (0, 1)
        ]
        # comm buffers
        hx_self = const_p.tile([P, N_RANGES, TXCOL], BF16, tag="hxself", name="hx_self")
        hx_land = const_p.tile(
            [P, N_CORES - 1, N_RANGES, TXCOL], BF16, tag="hxland", name="hx_land"
        )
        nc.vector.memset(hx_self[:, :, DIM + 2 : DIM + 4], 1.0)

        # zero h1pad tail rows once
        zpad = const_p.tile([P, DIM], BF16, tag="zpad")
        nc.vector.memset(zpad[:], 0.0)
        nc.sync.dma_start(
            out=h1pad[SHARD : N_RANGES * P, :], in_=zpad[: N_RANGES * P - SHARD, :]
        )

        def phase_A(L, lhsT_fn, prefetch=None, on_range=None):
            for nt in range(N_RANGES):
                pre = prefetch(nt) if prefetch is not None else None
                ps = ppA.tile([P, DIM + 2], F32, tag="psA")
                for k in range(KT):
                    lhsT = lhsT_fn(k, nt, pre)
                    rhs = Wv_sb[L][:, k, :]
                    nc.tensor.matmul(
                        out=ps[:, 0:512], lhsT=lhsT, rhs=rhs[:, 0:512],
                        start=(k == 0), stop=(k == KT - 1),
                    )
                    nc.tensor.matmul(
                        out=ps[:, 512 : DIM + 2], lhsT=lhsT, rhs=rhs[:, 512 : DIM + 2],
                        start=(k == 0), stop=(k == KT - 1),
                    )
                nc.vector.tensor_copy(out=hx_self[:, nt, 0:DIM], in_=ps[:, 0:DIM])
                nc.vector.tensor_copy(
                    out=hx_self[:, nt, ACOL : ACOL + 2].bitcast(F32),
                    in_=ps[:, DIM : DIM + 1],
                )
                nc.vector.tensor_copy(
                    out=adcol[L][:, nt : nt + 1], in_=ps[:, DIM + 1 : DIM + 2]
                )
                if on_range is not None:
                    on_range(nt)

        NG = ng  # broadcast groups per layer
        GR = N_RANGES // NG
        PREPS_L = 7 * NG + 1  # prep_sem incs per layer
        LOC_L = 16 * 7 * NG + 16  # loc_sem incs per layer

        def bcast_group(L, idx, grp):
            """Broadcast hx_self ranges [grp*GR, (grp+1)*GR) to all peers."""
            rsl = slice(grp * GR, (grp + 1) * GR)
            with tc.tile_critical(name=f"bc{L}_{grp}"):
                g = nc.gpsimd
                for s in range(1, N_CORES):
                    rdests = [(0, s) if k == s else None for k in range(N_CORES)]
                    g.remote_dma_broadcast(
                        out_ap=hx_land[:, s - 1, rsl],
                        in_ap=hx_self[:, rsl],
                        remote_sem=arr_sems[s - 1],
                        local_sem=loc_sem,
                        rdests=rdests,
                    ).then_inc(prep_sem, 1)
                g.wait_ge(prep_sem, PREPS_L * idx + 7 * (grp + 1))
                if grp == 0 and idx > 0:
                    g.wait_ge(ack_sem, 14 * idx)  # peers drained their landings
                g.trigger_dma(count=7)

        def spill_block(L, idx):
            def spill(s):
                src = hx_self[:] if s == 0 else hx_land[:, s - 1]
                dst = hx_full[L][
                    s * SLOT_ROWS : (s + 1) * SLOT_ROWS, 0:TXCOL
                ].rearrange("(r p) c -> p r c", p=P)
                nc.gpsimd.dma_start(out=dst, in_=src).then_inc(spill_sem, 16)

            with tc.tile_critical(name=f"sp{L}"):
                g = nc.gpsimd
                spill(0)  # own rows need no arrival wait
                for s in range(1, N_CORES):
                    g.wait_ge(arr_sems[s - 1], 2 * NG * (idx + 1))
                    spill(s)
                g.wait_ge(loc_sem, LOC_L * idx + 16 * 7 * NG)
                g.wait_ge(spill_sem, 128 * (idx + 1))
                ack_dests = [(0, k) if k != 0 else None for k in range(N_CORES)]
                g.remote_sem_update_broadcast(
                    remote_sem=ack_sem, local_sem=loc_sem, rdests=ack_dests
                ).then_inc(prep_sem, 1)
                g.wait_ge(prep_sem, PREPS_L * (idx + 1))
                g.trigger_dma(count=1)

        def phase_D(L, epilogue_fn):
            tile_base = 0
            for r in range(N_RANGES):
                T_r = tiles_per_range[r]
                ps_tr = ppT.tile([P, P], F32, tag="adtr")
                nc.tensor.transpose(
                    out=ps_tr[:],
                    in_=adcol[L][:, r : r + 1].to_broadcast([P, P]),
                    identity=identity[:],
                )
                adb = sb.tile([P, P], F32, tag="adb")
                nc.vector.tensor_copy(out=adb[:], in_=ps_tr[:])

                ps = pp.tile([P, DIM + 3], F32, tag="ps")
                for c0 in range(0, T_r, CHUNK_T):
                    ct = min(CHUNK_T, T_r - c0)
                    slot0 = (tile_base + c0) * P
                    G = gp.tile([P, CHUNK_T, ROW], BF16, tag="G")
                    nc.gpsimd.dma_gather(
                        out_ap=G[:, 0:ct, :],
                        in_ap=hx_full[L][:],
                        idxs_ap=idx_sb[:, slot0 // 16 : (slot0 + ct * P) // 16],
                        num_idxs=ct * P,
                        num_idxs_reg=ct * P,
                        elem_size=ROW,
                    )
                    pen_c = sb2.tile([P, CHUNK_T * P], BF16, tag="penc")
                    nc.sync.dma_start(
                        out=pen_c[:, 0 : ct * P],
                        in_=pen_d[:, slot0 : slot0 + ct * P],
                    )
                    Tb = sb2.tile([P, CHUNK_T * P], F32, tag="Tb")
                    E1 = sb2.tile([P, CHUNK_T * P], F32, tag="E1")
                    Sb = sb2.tile([P, CHUNK_T * P], BF16, tag="Sb")
                    for i in range(ct):
                        sl = slice(i * P, (i + 1) * P)
                        nc.vector.tensor_scalar(
                            out=Tb[:, sl],
                            in0=adb[:],
                            scalar1=G[:, i, ACOL : ACOL + 2].bitcast(F32),
                            scalar2=None,
                            op0=mybir.AluOpType.add,
                        )
                        nc.vector.tensor_tensor(
                            out=Tb[:, sl], in0=Tb[:, sl],
                            in1=pen_c[:, i * P : (i + 1) * P],
                            op=mybir.AluOpType.add,
                        )
                    w = slice(0, ct * P)
                    nc.scalar.activation(
                        out=E1[:, w], in_=Tb[:, w],
                        func=mybir.ActivationFunctionType.Exp,
                    )
                    nc.scalar.activation(
                        out=Tb[:, w], in_=Tb[:, w],
                        func=mybir.ActivationFunctionType.Exp, scale=NEG_SLOPE,
                    )
                    nc.vector.tensor_tensor(
                        out=Sb[:, w], in0=E1[:, w], in1=Tb[:, w],
                        op=mybir.AluOpType.max,
                    )
                    for i in range(ct):
                        first = c0 == 0 and i == 0
                        last = c0 + i == T_r - 1
                        sl = slice(i * P, (i + 1) * P)
                        nc.tensor.matmul(
                            out=ps[:, 0:512], lhsT=Sb[:, sl], rhs=G[:, i, 0:512],
                            start=first, stop=last,
                        )
                        nc.tensor.matmul(
                            out=ps[:, 512 : DIM + 3],
                            lhsT=Sb[:, sl], rhs=G[:, i, 512 : DIM + 3],
                            start=first, stop=last,
                        )
                epilogue_fn(r, ps)
                tile_base += T_r

        def xT_prefetch(nt):
            t = sb.tile([P, KT, P], BF16, tag="xTt")
            nc.sync.dma_start(
                out=t[:],
                in_=xT_d[:, nt * P : (nt + 1) * P].rearrange("(k p) n -> p k n", p=P),
            )
            return t

        for _rep in range(repeat):
            idx0 = 2 * _rep + 0
            idx1 = 2 * _rep + 1

            def a_hook(nt, L=0, idx=idx0):
                if (nt + 1) % GR == 0:
                    bcast_group(L, idx, (nt + 1) // GR - 1)

            phase_A(
                0, lambda k, nt, pre: pre[:, k, :], prefetch=xT_prefetch,
                on_range=a_hook,
            )
            spill_block(0, idx0)

            def epi1(r, ps):
                rows = _range_rows(r)
                rec = sb.tile([P, 1], F32, tag="rec")
                nc.vector.reciprocal(out=rec[:rows], in_=ps[:rows, DIM + 2 : DIM + 3])
                h1t = sb.tile([P, DIM], BF16, tag="h1t")
                nc.scalar.activation(
                    out=h1t[:rows], in_=ps[:rows, 0:DIM],
                    func=mybir.ActivationFunctionType.Relu, scale=rec[:rows],
                )
                nc.sync.dma_start(
                    out=h1pad[r * P : r * P + rows, :], in_=h1t[:rows, :]
                )

            phase_D(0, epi1)

            # ---------------- layer 2 ----------------
            def h1_prefetch(nt):
                t = sb.tile([P, KT, P], BF16, tag="h1Tt")
                for j in range(KT):
                    nc.sync.dma_start_transpose(
                        out=t[:, j, :],
                        in_=h1pad[nt * P : (nt + 1) * P, j * P : (j + 1) * P],
                    )
                return t

            phase_A(
                1, lambda k, nt, pre: pre[:, k, :], prefetch=h1_prefetch,
                on_range=lambda nt: a_hook(nt, L=1, idx=idx1),
            )
            spill_block(1, idx1)

            def epi2(r, ps):
                rows = _range_rows(r)
                rec = sb.tile([P, 1], F32, tag="rec")
                nc.vector.reciprocal(out=rec[:rows], in_=ps[:rows, DIM + 2 : DIM + 3])
                ot = sb.tile([P, DIM], F32, tag="ot")
                nc.scalar.activation(
                    out=ot[:rows], in_=ps[:rows, 0:DIM],
                    func=mybir.ActivationFunctionType.Copy, scale=rec[:rows],
                )
                nc.sync.dma_start(
                    out=out_d[r * P : r * P + rows, :], in_=ot[:rows, :]
                )

            phase_D(1, epi2)

    nc.compile()
    return nc


# ---------------------------------------------------------------------------
# entry point
# ---------------------------------------------------------------------------

_CACHE = {}
_PHI_PROBED = False


def _get_program(tiles_per_range):
    key = tuple(tiles_per_range)
    if key not in _CACHE:
        _CACHE[key] = build_program(tiles_per_range)
    return _CACHE[key]


def _ensure_phi():
    """Probe the physical NC pairing once; keep the static default on failure."""
    global PHI, _PHI_PROBED
    if _PHI_PROBED:
        return
    _PHI_PROBED = True
    try:
        PHI = _probe_phi()
    except Exception:
        pass


def kernel(x, edge_index, W1, att_src1, att_dst1, b1, W2, att_src2, att_dst2, b2):
    _ensure_phi()
    x = np.asarray(x, dtype=np.float32)
    edge_index = np.asarray(edge_index)
    in_maps, tiles_per_range = preprocess(
        x, edge_index,
        np.asarray(W1, np.float32), np.asarray(att_src1, np.float32),
        np.asarray(att_dst1, np.float32),
        np.asarray(W2, np.float32), np.asarray(att_src2, np.float32),
        np.asarray(att_dst2, np.float32),
    )
    b1 = np.asarray(b1, np.float32)
    b2 = np.asarray(b2, np.float32)
    if np.any(b1):
        raise NotImplementedError("nonzero b1 not supported by this kernel build")
    nc = _get_program(tiles_per_range)
    res = run_bass_kernel_spmd(nc, in_maps, list(range(N_CORES)))
    out = np.concatenate([res.results[c]["out"] for c in range(N_CORES)], axis=0)
    return (out + b2).astype(np.float32)


# revision 9
# speedup vs baseline: 3.6340x; 3.6340x over previous
"""Trainium2 Bass kernel for a 2-layer single-head GAT (PyG GATConv style).

Strategy (8 NeuronCores, graph/data parallel over destination nodes):
  - Host: add self-loops, shard edges by dst//1250, sort by dst, bucket into
    128-node dst ranges, pad each range's edge list to a core-uniform number
    of 128-edge tiles. Precompute per-edge gather indices (int16, dma_gather
    layout) and per-tile "penalty" masks (0 on the edge's dst column, -3000
    elsewhere -> exp() underflows masked entries to exactly 0).
  - Device, per layer:
      phase A: h||a_s||a_d = x_shard @ [W | W@att_src | W@att_dst]  (bf16)
      AllGather the (h bf16 || a_s fp32) rows -> hx_full [10000, 896] bf16
      phase D: per dst range: dma_gather rows of h[src] for its edges;
        S[e,n] = exp(lrelu(a_s[src_e] + a_d[n] + penalty[e,n]))  (lrelu via
        max(exp(x), exp(0.2x)));  PSUM[n, 0:768] += S^T @ G,
        PSUM[n, 768] += S^T @ 1  (softmax denominator, max-free: values are
        O(1) so exp cannot overflow);  epilogue: out = PSUM[:, :768] *
        (1/PSUM[:, 768]) (+bias) (+relu for layer 1).
  - Layer 2 input transposed via DMA-transpose (bf16) through DRAM.

The module builds one SPMD Bass program (identical for all 8 cores; only the
per-core input data differs) and runs it via run_bass_kernel_spmd.
"""

import math
import os
import sys
from contextlib import ExitStack

import numpy as np

for _p in ("/opt/trn_rl_repo", "/root/.axon_site/_ro/trn_rl_repo"):
    if os.path.isdir(_p) and _p not in sys.path:
        sys.path.insert(0, _p)

import ml_dtypes  # noqa: E402

import concourse.bass as bass  # noqa: E402
import concourse.tile as tile  # noqa: E402
from concourse import bacc, mybir  # noqa: E402
from concourse.bass_utils import run_bass_kernel_spmd  # noqa: E402
from concourse.masks import make_identity  # noqa: E402

F32 = mybir.dt.float32
F32R = mybir.dt.float32r
BF16 = mybir.dt.bfloat16
I16 = mybir.dt.int16

N_NODES = 10000
DIM = 768
N_CORES = 8
SHARD = N_NODES // N_CORES  # 1250
P = 128
N_RANGES = (SHARD + P - 1) // P  # 10 (last range has 98 nodes)
ROW = 896  # bf16 elems per gathered row (1792B, mult of 256)
ACOL = 770  # a_s stored as fp32 at bf16 cols [770:772]
NEG_SLOPE = 0.2
PENALTY = -3000.0
CHUNK_T = 8  # edge tiles per dma_gather chunk


def _range_rows(r):
    return min(P, SHARD - r * P)


# ---------------------------------------------------------------------------
# host preprocessing
# ---------------------------------------------------------------------------


def preprocess(x, edge_index, W1, att_src1, att_dst1, W2, att_src2, att_dst2):
    """Build per-core input maps + the tile structure (uniform across cores)."""
    n = x.shape[0]
    src = np.concatenate([np.asarray(edge_index[0]), np.arange(n, dtype=np.int64)])
    dst = np.concatenate([np.asarray(edge_index[1]), np.arange(n, dtype=np.int64)])

    # per (core, range) edge buckets
    core_of = dst // SHARD
    buckets = [[None] * N_RANGES for _ in range(N_CORES)]
    for c in range(N_CORES):
        sel = core_of == c
        s_c = src[sel]
        d_c = dst[sel] - c * SHARD
        order = np.argsort(d_c, kind="stable")
        s_c, d_c = s_c[order], d_c[order]
        rid = d_c // P
        for r in range(N_RANGES):
            m = rid == r
            buckets[c][r] = (s_c[m], (d_c[m] - r * P).astype(np.int64))

    tiles_per_range = [
        max(
            1,
            max((len(buckets[c][r][0]) + P - 1) // P for c in range(N_CORES)),
        )
        for r in range(N_RANGES)
    ]
    total_tiles = sum(tiles_per_range)
    total_slots = total_tiles * P

    Wv1 = np.concatenate(
        [W1, (W1 @ att_src1)[:, None], (W1 @ att_dst1)[:, None]], axis=1
    ).astype(np.float32)
    Wv2 = np.concatenate(
        [W2, (W2 @ att_src2)[:, None], (W2 @ att_dst2)[:, None]], axis=1
    ).astype(np.float32)

    in_maps = []
    for c in range(N_CORES):
        idx_slots = np.zeros(total_slots, dtype=np.int16)
        rel_slots = np.full(total_slots, -1, dtype=np.int32)
        off = 0
        for r in range(N_RANGES):
            s_r, rel_r = buckets[c][r]
            k = len(s_r)
            idx_slots[off : off + k] = s_r.astype(np.int16)
            rel_slots[off : off + k] = rel_r
            off += tiles_per_range[r] * P
        # dma_gather index layout: index i -> [partition i%16, slot i//16],
        # replicated across the 8 groups of 16 partitions.
        idx16 = idx_slots.reshape(-1, 16).T  # [16, total_slots/16]
        idx16 = np.tile(idx16, (8, 1)).copy()  # [128, total_slots/16]
        # penalty tiles: [p, t*128 + n] = 0 if rel[t*128+p] == n else PENALTY
        rel = rel_slots.reshape(total_tiles, P)  # [t, p]
        pen = np.where(
            rel[:, :, None] == np.arange(P)[None, None, :], 0.0, PENALTY
        )  # [t, p, n]
        pen = np.ascontiguousarray(pen.transpose(1, 0, 2).reshape(P, total_tiles * P))
        xT = np.zeros((DIM, N_RANGES * P), dtype=ml_dtypes.bfloat16)
        xT[:, :SHARD] = np.asarray(x[c * SHARD : (c + 1) * SHARD]).T.astype(ml_dtypes.bfloat16)
        in_maps.append(
            {
                "xT": xT,
                "Wv1": Wv1.astype(ml_dtypes.bfloat16),
                "Wv2": Wv2.astype(ml_dtypes.bfloat16),
                "idx": idx16.astype(np.int16),
                "pen": pen.astype(ml_dtypes.bfloat16),
            }
        )
    return in_maps, tiles_per_range


# ---------------------------------------------------------------------------
# device program
# ---------------------------------------------------------------------------


def build_program(tiles_per_range, debug_dumps=False, repeat=1, variant="full"):
    """variant: 'full' | 'nocc' (skip collectives) | 'cconly' (only collectives)
    | 'aonly' (phase A L1 only) | 'donly' (phase D L1 only, garbage input)."""
    do_a = variant in ("full", "nocc", "cconly", "aonly")
    do_cc = variant in ("full", "cconly")
    do_d = variant in ("full", "nocc", "donly")
    do_l2 = variant in ("full", "nocc", "cconly")
    total_tiles = sum(tiles_per_range)
    total_slots = total_tiles * P

    nc = bacc.Bacc(
        "TRN2",
        target_bir_lowering=False,
        debug=False,
        num_devices=N_CORES,
    )

    xT_d = nc.dram_tensor("xT", [DIM, N_RANGES * P], BF16, kind="ExternalInput")
    Wv1_d = nc.dram_tensor("Wv1", [DIM, DIM + 2], BF16, kind="ExternalInput")
    Wv2_d = nc.dram_tensor("Wv2", [DIM, DIM + 2], BF16, kind="ExternalInput")
    idx_d = nc.dram_tensor("idx", [P, total_slots // 16], I16, kind="ExternalInput")
    pen_d = nc.dram_tensor("pen", [P, total_tiles * P], BF16, kind="ExternalInput")
    out_d = nc.dram_tensor("out", [SHARD, DIM], F32, kind="ExternalOutput")

    hx_in = [nc.dram_tensor(f"hx{L}_in", [SHARD, ROW], BF16) for L in (1, 2)]
    hx_full = [
        nc.dram_tensor(f"hx{L}_full", [N_NODES, ROW], BF16, addr_space="Shared")
        for L in (1, 2)
    ]
    h1pad = nc.dram_tensor("h1pad", [N_RANGES * P, DIM], BF16)
    if debug_dumps:
        d_hx1in = nc.dram_tensor("d_hx1in", [SHARD, ROW], BF16, kind="ExternalOutput")
        d_hx1full = nc.dram_tensor("d_hx1full", [N_NODES, ROW], BF16, kind="ExternalOutput")
        d_h1pad = nc.dram_tensor("d_h1pad", [N_RANGES * P, DIM], BF16, kind="ExternalOutput")

    replica_groups = [list(range(N_CORES))]
    KT = DIM // P  # 6 k-tiles

    with tile.TileContext(nc) as tc, ExitStack() as ctx:
        const_p = ctx.enter_context(tc.tile_pool(name="const", bufs=1))
        sb = ctx.enter_context(tc.tile_pool(name="sb", bufs=3))
        sb2 = ctx.enter_context(tc.tile_pool(name="sb2", bufs=2))
        gp = ctx.enter_context(tc.tile_pool(name="gath", bufs=2))
        pp = ctx.enter_context(tc.tile_pool(name="psum", bufs=2, space="PSUM"))
        ppA = ctx.enter_context(tc.tile_pool(name="psumA", bufs=1, space="PSUM"))
        ppT = ctx.enter_context(tc.tile_pool(name="psumT", bufs=2, space="PSUM"))

        # resident constants
        identity = const_p.tile([P, P], F32)
        make_identity(nc, identity[:])
        ones_col = const_p.tile([P, 1], BF16)
        nc.vector.memset(ones_col[:], 1.0)
        idx_sb = const_p.tile([P, total_slots // 16], I16)
        nc.sync.dma_start(out=idx_sb[:], in_=idx_d[:])
        Wv_sb = [
            const_p.tile([P, KT, DIM + 2], BF16, tag="wv0", name="wv0"),
            const_p.tile([P, KT, DIM + 2], BF16, tag="wv1", name="wv1"),
        ]
        for L, wd in enumerate((Wv1_d, Wv2_d)):
            for k in range(KT):
                nc.sync.dma_start(
                    out=Wv_sb[L][:, k, :], in_=wd[k * P : (k + 1) * P, :]
                )
        adcol = [const_p.tile([P, N_RANGES], F32, tag=f"ad{L}", name=f"adcol{L}") for L in (0, 1)]
        xT_all = const_p.tile([P, KT, N_RANGES * P], BF16)
        nc.sync.dma_start(
            out=xT_all[:], in_=xT_d[:].rearrange("(k p) n -> p k n", p=P)
        )

        # zero the h1pad tail rows once (they feed junk lhsT columns otherwise)
        zpad = const_p.tile([P, DIM], BF16, tag="zpad")
        nc.vector.memset(zpad[:], 0.0)
        nc.sync.dma_start(
            out=h1pad[SHARD : N_RANGES * P, :], in_=zpad[: N_RANGES * P - SHARD, :]
        )

        def phase_A(L, lhsT_tile_fn):
            """node transform: psum[128, 770] = x_tile @ [W | vs | vd]."""
            for nt in range(N_RANGES):
                ps = ppA.tile([P, DIM + 2], F32, tag="psA")
                for k in range(KT):
                    lhsT = lhsT_tile_fn(k, nt)
                    rhs = Wv_sb[L][:, k, :]
                    nc.tensor.matmul(
                        out=ps[:, 0:512],
                        lhsT=lhsT,
                        rhs=rhs[:, 0:512],
                        start=(k == 0),
                        stop=(k == KT - 1),
                    )
                    nc.tensor.matmul(
                        out=ps[:, 512 : DIM + 2],
                        lhsT=lhsT,
                        rhs=rhs[:, 512 : DIM + 2],
                        start=(k == 0),
                        stop=(k == KT - 1),
                    )
                rows = _range_rows(nt)
                hxt = sb.tile([P, ROW], BF16, tag="hxt")
                nc.vector.memset(hxt[:, DIM:ROW], 0.0)
                nc.vector.memset(hxt[:, ACOL + 2 : ACOL + 3], 1.0)
                nc.vector.tensor_copy(out=hxt[:, 0:DIM], in_=ps[:, 0:DIM])
                nc.vector.tensor_copy(
                    out=hxt[:, ACOL : ACOL + 2].bitcast(F32),
                    in_=ps[:, DIM : DIM + 1],
                )
                nc.vector.tensor_copy(
                    out=adcol[L][:, nt : nt + 1], in_=ps[:, DIM + 1 : DIM + 2]
                )
                nc.sync.dma_start(
                    out=hx_in[L][nt * P : nt * P + rows, :], in_=hxt[:rows, :]
                )

        def phase_D(L, epilogue_fn):
            """gather + masked-softmax scatter matmul, one dst range at a time."""
            tile_base = 0
            for r in range(N_RANGES):
                T_r = tiles_per_range[r]
                # a_d broadcast [e, n] = a_d[base + n] via PE transpose
                ps_tr = ppT.tile([P, P], F32, tag="adtr")
                nc.tensor.transpose(
                    out=ps_tr[:],
                    in_=adcol[L][:, r : r + 1].to_broadcast([P, P]),
                    identity=identity[:],
                )
                adb = sb.tile([P, P], F32, tag="adb")
                nc.vector.tensor_copy(out=adb[:], in_=ps_tr[:])

                ps = pp.tile([P, DIM + 5], F32, tag="ps")
                for c0 in range(0, T_r, CHUNK_T):
                    ct = min(CHUNK_T, T_r - c0)
                    slot0 = (tile_base + c0) * P
                    G = gp.tile([P, CHUNK_T, ROW], BF16, tag="G")
                    nc.gpsimd.dma_gather(
                        out_ap=G[:, 0:ct, :],
                        in_ap=hx_full[L][:],
                        idxs_ap=idx_sb[:, slot0 // 16 : (slot0 + ct * P) // 16],
                        num_idxs=ct * P,
                        num_idxs_reg=ct * P,
                        elem_size=ROW,
                    )
                    pen_c = sb2.tile([P, CHUNK_T * P], BF16, tag="penc")
                    nc.sync.dma_start(
                        out=pen_c[:, 0 : ct * P],
                        in_=pen_d[:, slot0 : slot0 + ct * P],
                    )
                    Tb = sb2.tile([P, CHUNK_T * P], F32, tag="Tb")
                    E1 = sb2.tile([P, CHUNK_T * P], F32, tag="E1")
                    Sb = sb2.tile([P, CHUNK_T * P], BF16, tag="Sb")
                    for i in range(ct):
                        sl = slice(i * P, (i + 1) * P)
                        nc.vector.tensor_scalar(
                            out=Tb[:, sl],
                            in0=adb[:],
                            scalar1=G[:, i, ACOL : ACOL + 2].bitcast(F32),
                            scalar2=None,
                            op0=mybir.AluOpType.add,
                        )
                        nc.vector.tensor_tensor(
                            out=Tb[:, sl],
                            in0=Tb[:, sl],
                            in1=pen_c[:, i * P : (i + 1) * P],
                            op=mybir.AluOpType.add,
                        )
                    w = slice(0, ct * P)
                    # exp(lrelu(x)) = max(exp(x), exp(0.2 x))
                    nc.scalar.activation(
                        out=E1[:, w], in_=Tb[:, w],
                        func=mybir.ActivationFunctionType.Exp,
                    )
                    nc.scalar.activation(
                        out=Tb[:, w], in_=Tb[:, w],
                        func=mybir.ActivationFunctionType.Exp, scale=NEG_SLOPE,
                    )
                    nc.vector.tensor_tensor(
                        out=Sb[:, w], in0=E1[:, w], in1=Tb[:, w],
                        op=mybir.AluOpType.max,
                    )
                    for i in range(ct):
                        first = c0 == 0 and i == 0
                        last = c0 + i == T_r - 1
                        sl = slice(i * P, (i + 1) * P)
                        nc.tensor.matmul(
                            out=ps[:, 0:512], lhsT=Sb[:, sl], rhs=G[:, i, 0:512],
                            start=first, stop=last,
                        )
                        nc.tensor.matmul(
                            out=ps[:, 512 : DIM + 5],
                            lhsT=Sb[:, sl], rhs=G[:, i, 512 : ACOL + 3],
                            start=first, stop=last,
                        )
                epilogue_fn(r, ps)
                tile_base += T_r

        # ---------------- layer 1 ----------------
        for _rep in range(repeat):
            if do_a:
                phase_A(0, lambda k, nt: xT_all[:, k, nt * P : (nt + 1) * P])
            if do_cc:
                nc.gpsimd.collective_compute(
                    "AllGather",
                    mybir.AluOpType.bypass,
                    replica_groups=replica_groups,
                    ins=[hx_in[0][:]],
                    outs=[hx_full[0][:]],
                )

            if debug_dumps:
                nc.sync.dma_start(out=d_hx1in[:], in_=hx_in[0][:])
                nc.sync.dma_start(out=d_hx1full[:], in_=hx_full[0][:])

            def epi1(r, ps):
                rows = _range_rows(r)
                rec = sb.tile([P, 1], F32, tag="rec")
                nc.vector.reciprocal(out=rec[:rows], in_=ps[:rows, ACOL + 2 : ACOL + 3])
                h1t = sb.tile([P, DIM], BF16, tag="h1t")
                nc.scalar.activation(
                    out=h1t[:rows], in_=ps[:rows, 0:DIM],
                    func=mybir.ActivationFunctionType.Relu, scale=rec[:rows],
                )
                nc.sync.dma_start(out=h1pad[r * P : r * P + rows, :], in_=h1t[:rows, :])

            if do_d:
                phase_D(0, epi1)
            if debug_dumps:
                nc.sync.dma_start(out=d_h1pad[:], in_=h1pad[:])

            # ---------------- layer 2 ----------------
            if do_l2:
                h1T = [const_p.tile([P, N_RANGES * P], BF16, tag=f"h1T{j}", name=f"h1T{j}") for j in range(KT)]
                for j in range(KT):
                    nc.sync.dma_start_transpose(
                        out=h1T[j][:], in_=h1pad[:, j * P : (j + 1) * P]
                    )
                if do_a:
                    phase_A(1, lambda k, nt: h1T[k][:, nt * P : (nt + 1) * P])
                if do_cc:
                    nc.gpsimd.collective_compute(
                        "AllGather",
                        mybir.AluOpType.bypass,
                        replica_groups=replica_groups,
                        ins=[hx_in[1][:]],
                        outs=[hx_full[1][:]],
                    )

            def epi2(r, ps):
                rows = _range_rows(r)
                rec = sb.tile([P, 1], F32, tag="rec")
                nc.vector.reciprocal(out=rec[:rows], in_=ps[:rows, ACOL + 2 : ACOL + 3])
                ot = sb.tile([P, DIM], F32, tag="ot")
                nc.scalar.activation(
                    out=ot[:rows], in_=ps[:rows, 0:DIM],
                    func=mybir.ActivationFunctionType.Copy, scale=rec[:rows],
                )
                nc.sync.dma_start(out=out_d[r * P : r * P + rows, :], in_=ot[:rows, :])

            if do_d and do_l2:
                phase_D(1, epi2)

    nc.compile()
    return nc


# ---------------------------------------------------------------------------
# entry point
# ---------------------------------------------------------------------------

_CACHE = {}


def _get_program(tiles_per_range):
    key = tuple(tiles_per_range)
    if key not in _CACHE:
        _CACHE[key] = build_program(tiles_per_range)
    return _CACHE[key]


def kernel(x, edge_index, W1, att_src1, att_dst1, b1, W2, att_src2, att_dst2, b2):
    x = np.asarray(x, dtype=np.float32)
    edge_index = np.asarray(edge_index)
    in_maps, tiles_per_range = preprocess(
        x, edge_index,
        np.asarray(W1, np.float32), np.asarray(att_src1, np.float32),
        np.asarray(att_dst1, np.float32),
        np.asarray(W2, np.float32), np.asarray(att_src2, np.float32),
        np.asarray(att_dst2, np.float32),
    )
    b1 = np.asarray(b1, np.float32)
    b2 = np.asarray(b2, np.float32)
    if np.any(b1):
        raise NotImplementedError("nonzero b1 not supported by this kernel build")
    nc = _get_program(tiles_per_range)
    res = run_bass_kernel_spmd(nc, in_maps, list(range(N_CORES)))
    out = np.concatenate([res.results[c]["out"] for c in range(N_CORES)], axis=0)
    return (out + b2).astype(np.float32)



# revision 16
# speedup vs baseline: 4.6253x; 1.2728x over previous
"""Trainium2 Bass kernel for a 2-layer single-head GAT (PyG GATConv style).

Strategy (8 NeuronCores, graph/data parallel over destination nodes):
  - Host: add self-loops, shard edges by dst//1250, sort by dst, bucket into
    128-node dst ranges, pad each range's edge list to a core-uniform number
    of 128-edge tiles. Precompute per-edge gather indices (int16, dma_gather
    layout) and per-tile "penalty" masks (0 on the edge's dst column, -3000
    elsewhere -> exp() underflows masked entries to exactly 0).
  - Device, per layer:
      phase A: h||a_s||a_d = x_shard @ [W | W@att_src | W@att_dst]  (bf16)
      AllGather the (h bf16 || a_s fp32) rows -> hx_full [10000, 896] bf16
      phase D: per dst range: dma_gather rows of h[src] for its edges;
        S[e,n] = exp(lrelu(a_s[src_e] + a_d[n] + penalty[e,n]))  (lrelu via
        max(exp(x), exp(0.2x)));  PSUM[n, 0:768] += S^T @ G,
        PSUM[n, 768] += S^T @ 1  (softmax denominator, max-free: values are
        O(1) so exp cannot overflow);  epilogue: out = PSUM[:, :768] *
        (1/PSUM[:, 768]) (+bias) (+relu for layer 1).
  - Layer 2 input transposed via DMA-transpose (bf16) through DRAM.

The module builds one SPMD Bass program (identical for all 8 cores; only the
per-core input data differs) and runs it via run_bass_kernel_spmd.
"""

import math
import os
import sys
from contextlib import ExitStack

import numpy as np

for _p in ("/opt/trn_rl_repo", "/root/.axon_site/_ro/trn_rl_repo"):
    if os.path.isdir(_p) and _p not in sys.path:
        sys.path.insert(0, _p)

import ml_dtypes  # noqa: E402

import concourse.bass as bass  # noqa: E402
import concourse.tile as tile  # noqa: E402
from concourse import bacc, mybir  # noqa: E402
from concourse.bass_utils import run_bass_kernel_spmd  # noqa: E402
from concourse.masks import make_identity  # noqa: E402

F32 = mybir.dt.float32
F32R = mybir.dt.float32r
BF16 = mybir.dt.bfloat16
I16 = mybir.dt.int16

N_NODES = 10000
DIM = 768
N_CORES = 8
SHARD = N_NODES // N_CORES  # 1250
P = 128
N_RANGES = (SHARD + P - 1) // P  # 10 (last range has 98 nodes)
ROW = 896  # bf16 elems per gathered row (1792B, mult of 256)
ACOL = 770  # a_s stored as fp32 at bf16 cols [770:772]
NEG_SLOPE = 0.2
PENALTY = -3000.0
CHUNK_T = 8  # edge tiles per dma_gather chunk
HALF = SHARD // 2  # 625; AllGather is split into two half-collectives


def _range_rows(r):
    return min(P, SHARD - r * P)


# ---------------------------------------------------------------------------
# host preprocessing
# ---------------------------------------------------------------------------


def preprocess(x, edge_index, W1, att_src1, att_dst1, W2, att_src2, att_dst2):
    """Build per-core input maps + the tile structure (uniform across cores)."""
    n = x.shape[0]
    src = np.concatenate([np.asarray(edge_index[0]), np.arange(n, dtype=np.int64)])
    dst = np.concatenate([np.asarray(edge_index[1]), np.arange(n, dtype=np.int64)])

    # per (core, range) edge buckets
    core_of = dst // SHARD
    buckets = [[None] * N_RANGES for _ in range(N_CORES)]
    for c in range(N_CORES):
        sel = core_of == c
        s_c = src[sel]
        d_c = dst[sel] - c * SHARD
        # hx_full row layout: [half][core][625] so each half-AllGather
        # writes a contiguous block
        sc_core = s_c // SHARD
        sc_loc = s_c % SHARD
        g_c = np.where(
            sc_loc < HALF,
            sc_core * HALF + sc_loc,
            N_CORES * HALF + sc_core * HALF + (sc_loc - HALF),
        )
        order = np.argsort(d_c, kind="stable")
        g_c, d_c = g_c[order], d_c[order]
        rid = d_c // P
        for r in range(N_RANGES):
            m = rid == r
            buckets[c][r] = (g_c[m], (d_c[m] - r * P).astype(np.int64))

    tiles_per_range = [
        max(
            1,
            max((len(buckets[c][r][0]) + P - 1) // P for c in range(N_CORES)),
        )
        for r in range(N_RANGES)
    ]
    total_tiles = sum(tiles_per_range)
    total_slots = total_tiles * P

    Wv1 = np.concatenate(
        [W1, (W1 @ att_src1)[:, None], (W1 @ att_dst1)[:, None]], axis=1
    ).astype(np.float32)
    Wv2 = np.concatenate(
        [W2, (W2 @ att_src2)[:, None], (W2 @ att_dst2)[:, None]], axis=1
    ).astype(np.float32)

    in_maps = []
    for c in range(N_CORES):
        idx_slots = np.zeros(total_slots, dtype=np.int16)
        rel_slots = np.full(total_slots, -1, dtype=np.int32)
        off = 0
        for r in range(N_RANGES):
            s_r, rel_r = buckets[c][r]
            k = len(s_r)
            idx_slots[off : off + k] = s_r.astype(np.int16)
            rel_slots[off : off + k] = rel_r
            off += tiles_per_range[r] * P
        # dma_gather index layout: index i -> [partition i%16, slot i//16],
        # replicated across the 8 groups of 16 partitions.
        idx16 = idx_slots.reshape(-1, 16).T  # [16, total_slots/16]
        idx16 = np.tile(idx16, (8, 1)).copy()  # [128, total_slots/16]
        # penalty tiles: [p, t*128 + n] = 0 if rel[t*128+p] == n else PENALTY
        rel = rel_slots.reshape(total_tiles, P)  # [t, p]
        pen = np.where(
            rel[:, :, None] == np.arange(P)[None, None, :], 0.0, PENALTY
        )  # [t, p, n]
        pen = np.ascontiguousarray(pen.transpose(1, 0, 2).reshape(P, total_tiles * P))
        xT = np.zeros((DIM, N_RANGES * P), dtype=ml_dtypes.bfloat16)
        xT[:, :SHARD] = np.asarray(x[c * SHARD : (c + 1) * SHARD]).T.astype(ml_dtypes.bfloat16)
        in_maps.append(
            {
                "xT": xT,
                "Wv1": Wv1.astype(ml_dtypes.bfloat16),
                "Wv2": Wv2.astype(ml_dtypes.bfloat16),
                "idx": idx16.astype(np.int16),
                "pen": pen.astype(ml_dtypes.bfloat16),
            }
        )
    return in_maps, tiles_per_range


# ---------------------------------------------------------------------------
# device program
# ---------------------------------------------------------------------------


def build_program(tiles_per_range, debug_dumps=False, repeat=1, variant="full"):
    """variant: 'full' | 'nocc' (skip collectives) | 'cconly' (only collectives)
    | 'aonly' (phase A L1 only) | 'donly' (phase D L1 only, garbage input)."""
    do_a = variant in ("full", "nocc", "cconly", "aonly")
    do_cc = variant in ("full", "cconly")
    do_d = variant in ("full", "nocc", "donly")
    do_l2 = variant in ("full", "nocc", "cconly")
    total_tiles = sum(tiles_per_range)
    total_slots = total_tiles * P

    nc = bacc.Bacc(
        "TRN2",
        target_bir_lowering=False,
        debug=False,
        num_devices=N_CORES,
    )

    xT_d = nc.dram_tensor("xT", [DIM, N_RANGES * P], BF16, kind="ExternalInput")
    Wv1_d = nc.dram_tensor("Wv1", [DIM, DIM + 2], BF16, kind="ExternalInput")
    Wv2_d = nc.dram_tensor("Wv2", [DIM, DIM + 2], BF16, kind="ExternalInput")
    idx_d = nc.dram_tensor("idx", [P, total_slots // 16], I16, kind="ExternalInput")
    pen_d = nc.dram_tensor("pen", [P, total_tiles * P], BF16, kind="ExternalInput")
    out_d = nc.dram_tensor("out", [SHARD, DIM], F32, kind="ExternalOutput")

    hx_in = [nc.dram_tensor(f"hx{L}_in", [SHARD, ROW], BF16) for L in (1, 2)]
    hx_full = [
        nc.dram_tensor(f"hx{L}_full", [N_NODES, ROW], BF16, addr_space="Shared")
        for L in (1, 2)
    ]
    h1pad = nc.dram_tensor("h1pad", [N_RANGES * P, DIM], BF16)
    if debug_dumps:
        d_hx1in = nc.dram_tensor("d_hx1in", [SHARD, ROW], BF16, kind="ExternalOutput")
        d_hx1full = nc.dram_tensor("d_hx1full", [N_NODES, ROW], BF16, kind="ExternalOutput")
        d_h1pad = nc.dram_tensor("d_h1pad", [N_RANGES * P, DIM], BF16, kind="ExternalOutput")

    replica_groups = [list(range(N_CORES))]
    KT = DIM // P  # 6 k-tiles

    with tile.TileContext(nc) as tc, ExitStack() as ctx:
        const_p = ctx.enter_context(tc.tile_pool(name="const", bufs=1))
        sb = ctx.enter_context(tc.tile_pool(name="sb", bufs=3))
        sb2 = ctx.enter_context(tc.tile_pool(name="sb2", bufs=3))
        gp = ctx.enter_context(tc.tile_pool(name="gath", bufs=4))
        pp = ctx.enter_context(tc.tile_pool(name="psum", bufs=2, space="PSUM"))
        ppA = ctx.enter_context(tc.tile_pool(name="psumA", bufs=1, space="PSUM"))
        ppT = ctx.enter_context(tc.tile_pool(name="psumT", bufs=2, space="PSUM"))

        # resident constants
        identity = const_p.tile([P, P], F32)
        make_identity(nc, identity[:])
        ones_col = const_p.tile([P, 1], BF16)
        nc.vector.memset(ones_col[:], 1.0)
        idx_sb = const_p.tile([P, total_slots // 16], I16)
        nc.sync.dma_start(out=idx_sb[:], in_=idx_d[:])
        Wv_sb = [
            const_p.tile([P, KT, DIM + 2], BF16, tag="wv0", name="wv0"),
            const_p.tile([P, KT, DIM + 2], BF16, tag="wv1", name="wv1"),
        ]
        for L, wd in enumerate((Wv1_d, Wv2_d)):
            for k in range(KT):
                nc.sync.dma_start(
                    out=Wv_sb[L][:, k, :], in_=wd[k * P : (k + 1) * P, :]
                )
        adcol = [const_p.tile([P, N_RANGES], F32, tag=f"ad{L}", name=f"adcol{L}") for L in (0, 1)]
        xT_all = const_p.tile([P, KT, N_RANGES * P], BF16)
        nc.sync.dma_start(
            out=xT_all[:], in_=xT_d[:].rearrange("(k p) n -> p k n", p=P)
        )

        # zero the h1pad tail rows once (they feed junk lhsT columns otherwise)
        zpad = const_p.tile([P, DIM], BF16, tag="zpad")
        nc.vector.memset(zpad[:], 0.0)
        nc.sync.dma_start(
            out=h1pad[SHARD : N_RANGES * P, :], in_=zpad[: N_RANGES * P - SHARD, :]
        )

        def phase_A(L, lhsT_tile_fn):
            """node transform: psum[128, 770] = x_tile @ [W | vs | vd]."""
            for nt in range(N_RANGES):
                ps = ppA.tile([P, DIM + 2], F32, tag="psA")
                for k in range(KT):
                    lhsT = lhsT_tile_fn(k, nt)
                    rhs = Wv_sb[L][:, k, :]
                    nc.tensor.matmul(
                        out=ps[:, 0:512],
                        lhsT=lhsT,
                        rhs=rhs[:, 0:512],
                        start=(k == 0),
                        stop=(k == KT - 1),
                    )
                    nc.tensor.matmul(
                        out=ps[:, 512 : DIM + 2],
                        lhsT=lhsT,
                        rhs=rhs[:, 512 : DIM + 2],
                        start=(k == 0),
                        stop=(k == KT - 1),
                    )
                rows = _range_rows(nt)
                hxt = sb.tile([P, ROW], BF16, tag="hxt")
                nc.vector.memset(hxt[:, DIM:ROW], 0.0)
                nc.vector.memset(hxt[:, ACOL + 2 : ACOL + 3], 1.0)
                nc.vector.tensor_copy(out=hxt[:, 0:DIM], in_=ps[:, 0:DIM])
                nc.vector.tensor_copy(
                    out=hxt[:, ACOL : ACOL + 2].bitcast(F32),
                    in_=ps[:, DIM : DIM + 1],
                )
                nc.vector.tensor_copy(
                    out=adcol[L][:, nt : nt + 1], in_=ps[:, DIM + 1 : DIM + 2]
                )
                nc.sync.dma_start(
                    out=hx_in[L][nt * P : nt * P + rows, :], in_=hxt[:rows, :]
                )

        def phase_D(L, epilogue_fn):
            """gather + masked-softmax scatter matmul, one dst range at a time."""
            tile_base = 0
            for r in range(N_RANGES):
                T_r = tiles_per_range[r]
                # a_d broadcast [e, n] = a_d[base + n] via PE transpose
                ps_tr = ppT.tile([P, P], F32, tag="adtr")
                nc.tensor.transpose(
                    out=ps_tr[:],
                    in_=adcol[L][:, r : r + 1].to_broadcast([P, P]),
                    identity=identity[:],
                )
                adb = sb.tile([P, P], F32, tag="adb")
                nc.vector.tensor_copy(out=adb[:], in_=ps_tr[:])

                ps = pp.tile([P, DIM + 5], F32, tag="ps")
                for c0 in range(0, T_r, CHUNK_T):
                    ct = min(CHUNK_T, T_r - c0)
                    slot0 = (tile_base + c0) * P
                    G = gp.tile([P, CHUNK_T, ROW], BF16, tag="G")
                    nc.gpsimd.dma_gather(
                        out_ap=G[:, 0:ct, :],
                        in_ap=hx_full[L][:],
                        idxs_ap=idx_sb[:, slot0 // 16 : (slot0 + ct * P) // 16],
                        num_idxs=ct * P,
                        num_idxs_reg=ct * P,
                        elem_size=ROW,
                    )
                    pen_c = sb2.tile([P, CHUNK_T * P], BF16, tag="penc")
                    nc.sync.dma_start(
                        out=pen_c[:, 0 : ct * P],
                        in_=pen_d[:, slot0 : slot0 + ct * P],
                    )
                    Tb = sb2.tile([P, CHUNK_T * P], F32, tag="Tb")
                    E1 = sb2.tile([P, CHUNK_T * P], F32, tag="E1")
                    Sb = sb2.tile([P, CHUNK_T * P], BF16, tag="Sb")
                    for i in range(ct):
                        sl = slice(i * P, (i + 1) * P)
                        nc.vector.tensor_scalar(
                            out=Tb[:, sl],
                            in0=adb[:],
                            scalar1=G[:, i, ACOL : ACOL + 2].bitcast(F32),
                            scalar2=None,
                            op0=mybir.AluOpType.add,
                        )
                        nc.vector.tensor_tensor(
                            out=Tb[:, sl],
                            in0=Tb[:, sl],
                            in1=pen_c[:, i * P : (i + 1) * P],
                            op=mybir.AluOpType.add,
                        )
                    w = slice(0, ct * P)
                    # exp(lrelu(x)) = max(exp(x), exp(0.2 x))
                    nc.scalar.activation(
                        out=E1[:, w], in_=Tb[:, w],
                        func=mybir.ActivationFunctionType.Exp,
                    )
                    nc.scalar.activation(
                        out=Tb[:, w], in_=Tb[:, w],
                        func=mybir.ActivationFunctionType.Exp, scale=NEG_SLOPE,
                    )
                    nc.vector.tensor_tensor(
                        out=Sb[:, w], in0=E1[:, w], in1=Tb[:, w],
                        op=mybir.AluOpType.max,
                    )
                    for i in range(ct):
                        first = c0 == 0 and i == 0
                        last = c0 + i == T_r - 1
                        sl = slice(i * P, (i + 1) * P)
                        nc.tensor.matmul(
                            out=ps[:, 0:512], lhsT=Sb[:, sl], rhs=G[:, i, 0:512],
                            start=first, stop=last,
                        )
                        nc.tensor.matmul(
                            out=ps[:, 512 : DIM + 5],
                            lhsT=Sb[:, sl], rhs=G[:, i, 512 : ACOL + 3],
                            start=first, stop=last,
                        )
                epilogue_fn(r, ps)
                tile_base += T_r

        # ---------------- layer 1 ----------------
        for _rep in range(repeat):
            if do_a:
                phase_A(0, lambda k, nt: xT_all[:, k, nt * P : (nt + 1) * P])
            if do_cc:
                for h in range(2):
                    nc.gpsimd.collective_compute(
                        "AllGather",
                        mybir.AluOpType.bypass,
                        replica_groups=replica_groups,
                        ins=[hx_in[0][h * HALF : (h + 1) * HALF, :]],
                        outs=[hx_full[0][h * N_CORES * HALF : (h + 1) * N_CORES * HALF, :]],
                    )

            if debug_dumps:
                nc.sync.dma_start(out=d_hx1in[:], in_=hx_in[0][:])
                nc.sync.dma_start(out=d_hx1full[:], in_=hx_full[0][:])

            def epi1(r, ps):
                rows = _range_rows(r)
                rec = sb.tile([P, 1], F32, tag="rec")
                nc.vector.reciprocal(out=rec[:rows], in_=ps[:rows, ACOL + 2 : ACOL + 3])
                h1t = sb.tile([P, DIM], BF16, tag="h1t")
                nc.scalar.activation(
                    out=h1t[:rows], in_=ps[:rows, 0:DIM],
                    func=mybir.ActivationFunctionType.Relu, scale=rec[:rows],
                )
                nc.sync.dma_start(out=h1pad[r * P : r * P + rows, :], in_=h1t[:rows, :])

            if do_d:
                phase_D(0, epi1)
            if debug_dumps:
                nc.sync.dma_start(out=d_h1pad[:], in_=h1pad[:])

            # ---------------- layer 2 ----------------
            if do_l2:
                h1T = [const_p.tile([P, N_RANGES * P], BF16, tag=f"h1T{j}", name=f"h1T{j}") for j in range(KT)]
                for j in range(KT):
                    nc.sync.dma_start_transpose(
                        out=h1T[j][:], in_=h1pad[:, j * P : (j + 1) * P]
                    )
                if do_a:
                    phase_A(1, lambda k, nt: h1T[k][:, nt * P : (nt + 1) * P])
                if do_cc:
                    for h in range(2):
                        nc.gpsimd.collective_compute(
                            "AllGather",
                            mybir.AluOpType.bypass,
                            replica_groups=replica_groups,
                            ins=[hx_in[1][h * HALF : (h + 1) * HALF, :]],
                            outs=[hx_full[1][h * N_CORES * HALF : (h + 1) * N_CORES * HALF, :]],
                        )

            def epi2(r, ps):
                rows = _range_rows(r)
                rec = sb.tile([P, 1], F32, tag="rec")
                nc.vector.reciprocal(out=rec[:rows], in_=ps[:rows, ACOL + 2 : ACOL + 3])
                ot = sb.tile([P, DIM], F32, tag="ot")
                nc.scalar.activation(
                    out=ot[:rows], in_=ps[:rows, 0:DIM],
                    func=mybir.ActivationFunctionType.Copy, scale=rec[:rows],
                )
                nc.sync.dma_start(out=out_d[r * P : r * P + rows, :], in_=ot[:rows, :])

            if do_d and do_l2:
                phase_D(1, epi2)

    nc.compile()
    return nc


# ---------------------------------------------------------------------------
# entry point
# ---------------------------------------------------------------------------

_CACHE = {}


def _get_program(tiles_per_range):
    key = tuple(tiles_per_range)
    if key not in _CACHE:
        _CACHE[key] = build_program(tiles_per_range)
    return _CACHE[key]


def kernel(x, edge_index, W1, att_src1, att_dst1, b1, W2, att_src2, att_dst2, b2):
    x = np.asarray(x, dtype=np.float32)
    edge_index = np.asarray(edge_index)
    in_maps, tiles_per_range = preprocess(
        x, edge_index,
        np.asarray(W1, np.float32), np.asarray(att_src1, np.float32),
        np.asarray(att_dst1, np.float32),
        np.asarray(W2, np.float32), np.asarray(att_src2, np.float32),
        np.asarray(att_dst2, np.float32),
    )
    b1 = np.asarray(b1, np.float32)
    b2 = np.asarray(b2, np.float32)
    if np.any(b1):
        raise NotImplementedError("nonzero b1 not supported by this kernel build")
    nc = _get_program(tiles_per_range)
    res = run_bass_kernel_spmd(nc, in_maps, list(range(N_CORES)))
    out = np.concatenate([res.results[c]["out"] for c in range(N_CORES)], axis=0)
    return (out + b2).astype(np.float32)

